# revision 32
# baseline (speedup 1.0000x reference)
"""Trainium2 Bass kernel for BaseXRayVolumeRenderer.

Full-input contract: kernel(**inputs) takes the unsharded inputs and returns
the full [1,1,256,256] output. Internally shards the 256x256 pixel grid
across 8 NeuronCores (4 row-blocks x 2 col-blocks).

Math: with R = I the trilinear sampling is separable per depth sample p:
    S_p = A_p^T @ (wz0*vol[z0] + wz1*vol[z1]) @ B_p
The z-blend is host-precomputed per depth sample (z0 is strictly increasing,
so each (z0, z0+1) slice pair belongs to exactly one p); the blended slice
and the A_p interp matrix are packed side by side in one "va" tensor, so
stage 1 is a single K=42 matmul per sample and stage 2 a single K=65 matmul
accumulating in PSUM.  Emission-absorption weights factorize into diagonal
scalings folded into A (sy/192) and B (sx), plus a per-block-of-8 rank-1
term G_p ~= u_p * v_b (u folded into B):
    gray = opac/4 + sum_b v_b * pacc_b.

Layout: frustum slicing - each core only loads the vol rows/cols its rays
touch (ny=42 of 128 y-rows, nx=65 of 128 x-cols): ~1.95MB HBM per core in
11 wave DMAs over three queues (vs 8.4MB in 28 DMAs for the two-matmul
z-gather layout).  Depth samples are processed in groups of 4 sharing one
PSUM tile; PSUM->SBUF f16 copies alternate between the vector and scalar
engines.

The global standardize+normalize reduces to out = (gray-gmin)/(gmax-gmin)
(the reference's 1e-8-epsilon terms contribute O(1e-9)).  In-kernel
AllReduce costs ~70us and remote_dma crashes on this platform, so per-core
per-row min/max go to the host, which combines 8x64 values and launches a
tiny second NEFF applying the affine to the f16 gray handoff.
"""

import numpy as np

import concourse.bass as bass
import concourse.bacc as bacc
import concourse.mybir as mybir
import concourse.tile as tile
from concourse.bass_utils import run_bass_kernel_spmd

F32 = mybir.dt.float32
F16 = mybir.dt.float16
F8 = mybir.dt.float8e4
ALU = mybir.AluOpType

IMG_H = 256
IMG_W = 256
N_PTS = 192
MIN_DEPTH, MAX_DEPTH, FOCAL = 3.0, 9.0, 4.0
EPS, EA_EPS = 1e-8, 1e-10
GRID = 128
N_CORES = 8
IB, JB = 64, 128            # per-core pixel block: 64 rows x 128 cols
BS = 8                      # depth-block size for the rank-1 absorption
NY, NX = 42, 65             # per-core vol window (y rows, x cols)
PB = 0                      # base partition for va/bt/py/ysb tiles
WARM_MM = 14                # warmup matmuls (cover first DMA wave, warm HAM)
P_WAVES = (0, 2, 8, 20, 40, 65)  # p-ranges of the DMA waves (tiny first)


def _interp_matrix(f):
    """f: [P, M] voxel coords -> [P, GRID, M] relu(1-|f-k|) interp weights."""
    k = np.arange(GRID, dtype=np.float64)[None, :, None]
    return np.maximum(0.0, 1.0 - np.abs(f[:, None, :] - k))


def _host_geometry(R, T):
    R = np.asarray(R, np.float64)
    T = np.asarray(T, np.float64)[0]
    assert np.allclose(R[0], np.eye(3), atol=1e-5), "kernel assumes R == I"
    ys = np.linspace(1.0, -1.0, IMG_H)
    xs = np.linspace(1.0, -1.0, IMG_W)
    d = np.linspace(MIN_DEPTH, MAX_DEPTH, N_PTS)
    fx = ((xs[None, :] * d[:, None] / FOCAL - T[0]) + 1.0) * 0.5 * (GRID - 1)
    fy = ((ys[None, :] * d[:, None] / FOCAL - T[1]) + 1.0) * 0.5 * (GRID - 1)
    fz = ((d - T[2]) + 1.0) * 0.5 * (GRID - 1)
    zf = np.floor(fz)
    wz = fz - zf
    z0 = np.clip(zf, 0, GRID - 1).astype(np.int64)
    wz0 = (1.0 - wz) * ((zf >= 0) & (zf <= GRID - 1))
    wz1 = wz * ((zf + 1 >= 0) & (zf + 1 <= GRID - 1))
    sz = wz0 + wz1
    active = np.nonzero(sz > 0)[0]
    assert len(active) and active[0] == 0 and np.all(np.diff(active) == 1), \
        "active depth samples must be a prefix for the prefix-cumprod fold"
    P = len(active)
    assert np.all(np.diff(z0[:P]) >= 1), "blend assumes strictly increasing z0"
    Ay = _interp_matrix(fy)[:P]          # [P, 128y, 256i]
    Bx = _interp_matrix(fx)[:P]          # [P, 128x, 256j]
    sy = Ay.sum(axis=1)                  # [P, 256]
    sx = Bx.sum(axis=1)
    dens = (sy[:, :, None] * sx[:, None, :]) * (sz[:P, None, None] / N_PTS)
    t = (1.0 + EA_EPS) - dens
    cp = np.cumprod(t, axis=0)
    absorption = np.concatenate([np.ones_like(cp[:1]), cp[:-1]], axis=0)
    opac4 = 0.25 * (1.0 - np.prod(1.0 - dens, axis=0))  # [H, W]
    # G_p = 0.75*sz_p*absorption_p ~= u_p * v_b  (rank-1 per block of BS)
    G = (0.75 * sz[:P, None, None] * absorption).reshape(P, -1)
    NB = (P + BS - 1) // BS
    u = np.zeros(P)
    v = np.zeros((NB, IMG_H * IMG_W))
    for b in range(NB):
        s, e = b * BS, min((b + 1) * BS, P)
        Ub, Sb, Vb = np.linalg.svd(G[s:e], full_matrices=False)
        sgn = np.sign(Ub[:, 0].mean()) or 1.0
        u[s:e] = Ub[:, 0] * Sb[0] * sgn
        v[b] = Vb[0] * sgn
    a_scale = sy / N_PTS                                  # [P, 256] (i)
    b_scale = sx * u[:, None]                             # [P, 256] (j)
    # per-block vol windows (rows: 4 blocks of 64, cols: 2 blocks of 128)
    row_wins, col_wins = [], []
    for r in range(4):
        nz = np.nonzero(Ay[:, :, r * IB:(r + 1) * IB].sum(axis=(0, 2)) > 0)[0]
        lo = min(int(nz[0]), GRID - NY)
        assert int(nz[-1]) < lo + NY
        row_wins.append(lo)
    for c in range(2):
        nz = np.nonzero(Bx[:, :, c * JB:(c + 1) * JB].sum(axis=(0, 2)) > 0)[0]
        lo = min(int(nz[0]), GRID - NX)
        assert int(nz[-1]) < lo + NX
        col_wins.append(lo)
    return dict(P=P, NB=NB, Ay=Ay, Bx=Bx, z0=[int(z) for z in z0[:P]],
                wz0=wz0[:P], wz1=wz1[:P], a_scale=a_scale, b_scale=b_scale,
                v=v.reshape(NB, IMG_H, IMG_W), opac4=opac4,
                row_wins=row_wins, col_wins=col_wins)


def _build_nc(P, NB):
    """Build the SPMD Bass program (geometry-independent: host pre-blends)."""
    nc = bacc.Bacc(num_devices=N_CORES)
    W = NX + IB                           # 129 cols per p in va
    va_d = nc.declare_dram_parameter("va", [NY, P * W], F16, isOutput=False)
    bt_d = nc.declare_dram_parameter("bt", [NX, P * JB], F16, isOutput=False)
    vb_d = nc.declare_dram_parameter("vb", [IB, (NB + 1) * JB], F16, isOutput=False)
    out_d = nc.declare_dram_parameter("out", [IB, JB + 4], F16, isOutput=True)

    with tile.TileContext(nc) as tc:
        with tc.tile_pool(name="big", bufs=1) as big:
            # partition placement: partitions 0..63 map to the 8 even SDMA
            # engines, 64..127 to the 8 odd ones. bt (stage-2 rhs, K=65,
            # forced to base 0) rides the evens; va and all pixel-row
            # tensors sit at base 64 so their DMAs ride the odds.
            va_sb = big.tile([64 + NY, P * W], F16)
            bt_sb = big.tile([NX, P * JB], F16)
            vb_sb = big.tile([64 + IB, (NB + 1) * JB], F16)
            gray_t = big.tile([64 + IB, JB], F32)
            gray = gray_t[64:64 + IB, :]
            gray16_t = big.tile([64 + IB, JB + 4], F16)

            # --- streamed loads: 4 waves x (va, bt) alternating HWDGE rings;
            # vb on the SWDGE ring.
            bt_rings = (nc.sync, nc.scalar, nc.sync, nc.scalar, nc.sync)
            va_rings = (nc.scalar, nc.sync, nc.gpsimd, nc.gpsimd, nc.scalar)
            for w in range(len(P_WAVES) - 1):
                p0, p1 = P_WAVES[w], P_WAVES[w + 1]
                va_rings[w].dma_start(va_sb[64:64 + NY, p0 * W:p1 * W],
                                      va_d[:, p0 * W:p1 * W])
                bt_rings[w].dma_start(bt_sb[:, p0 * JB:p1 * JB],
                                      bt_d[:, p0 * JB:p1 * JB])
            # vb is only needed from the first fold on; keep it behind the
            # early va waves in the SWDGE queue's FIFO
            nc.gpsimd.dma_start(vb_sb[64:64 + IB, :], vb_d[:])

            # --- main loop: groups of 4 depth samples share one PSUM tile.
            with tc.tile_pool(name="psY", bufs=3, space="PSUM") as psY, \
                 tc.tile_pool(name="psAcc", bufs=1, space="PSUM") as psAcc, \
                 tc.tile_pool(name="work", bufs=6) as work:
                nacc = (NB + 3) // 4
                pacct = [psAcc.tile([64 + IB, min(4, NB - 4 * a) * JB], F32,
                                    name=f"pacc{a}") for a in range(nacc)]
                paccs = [t[64:64 + IB, :] for t in pacct]

                def fold_block(b):
                    # gray += v_b * pacc_b (gray starts as opac/4); the
                    # final fold writes f16 straight into the output tile
                    pacc = paccs[b // 4]
                    col = (b % 4) * JB
                    tmp = work.tile([64 + IB, JB], F32, tag=f"tmp{b % 2}",
                                    name=f"tmpb{b}")
                    nc.vector.tensor_mul(tmp[64:64 + IB, :],
                                         pacc[:, col:col + JB],
                                         vb_sb[64:64 + IB,
                                               b * JB:(b + 1) * JB])
                    dst = gray16_t[64:64 + IB, 0:JB] if b == NB - 1 else gray
                    nc.vector.tensor_add(dst, gray, tmp[64:64 + IB, :])

                nc.vector.tensor_copy(gray, vb_sb[64:64 + IB, NB * JB:])
                groups = [list(range(s, min(s + 4, P))) for s in range(0, P, 4)]
                for gi, grp in enumerate(groups):
                    py = psY.tile([NX, len(grp) * IB], F32, tag="py",
                                  name=f"py{gi}")
                    for k, kk in enumerate(grp):
                        nc.tensor.matmul(
                            py[:, k * IB:(k + 1) * IB],
                            va_sb[64:64 + NY, kk * W:kk * W + NX],
                            va_sb[64:64 + NY, kk * W + NX:(kk + 1) * W],
                            start=True, stop=True,
                            tile_position=(64, 0))
                    ysb = work.tile([NX, len(grp) * IB], F16, tag="ysb",
                                    name=f"ysb{gi}")
                    if gi < 2 or gi % 2 == 1:
                        nc.vector.tensor_copy(ysb[:], py[:])
                    else:
                        nc.scalar.copy(ysb[:], py[:])
                    for k, kk in enumerate(grp):
                        b = kk // BS
                        pacc = paccs[b // 4]
                        col = (b % 4) * JB
                        first = (kk == b * BS)
                        last = (kk == min((b + 1) * BS, P) - 1)
                        nc.tensor.matmul(pacc[:, col:col + JB],
                                         ysb[:, k * IB:(k + 1) * IB],
                                         bt_sb[:, kk * JB:(kk + 1) * JB],
                                         start=first, stop=last,
                                         tile_position=(0, 64))
                        if last:
                            fold_block(b)

            # --- per-core per-row stats ride in 4 extra f16 columns of the
            # output (bit-cast f32 pairs); host reduces 8x64 values.
            with tc.tile_pool(name="st", bufs=1) as st:
                rowmm = st.tile([64 + IB, 2], F32)
                nc.vector.tensor_reduce(rowmm[64:64 + IB, 0:1],
                                        gray16_t[64:64 + IB, 0:JB],
                                        axis=mybir.AxisListType.X, op=ALU.min)
                nc.vector.tensor_reduce(rowmm[64:64 + IB, 1:2],
                                        gray16_t[64:64 + IB, 0:JB],
                                        axis=mybir.AxisListType.X, op=ALU.max)
                nc.vector.tensor_copy(gray16_t[64:64 + IB, JB:JB + 4],
                                      rowmm[64:64 + IB, :].bitcast(F16))
                nc.sync.dma_start(out_d[:], gray16_t[64:64 + IB, :])
    nc.finalize()
    return nc


def _build_affine():
    """Tiny second NEFF: out = a*gray + b per pixel (a,b host-reduced)."""
    nc = bacc.Bacc(num_devices=N_CORES)
    gray_d = nc.declare_dram_parameter("gray", [IB, JB + 4], F16, isOutput=False)
    out_d = nc.declare_dram_parameter("out", [IB, JB], F16, isOutput=True)
    with tile.TileContext(nc) as tc:
        with tc.tile_pool(name="aff", bufs=1) as pool:
            gsb = pool.tile([64 + IB, JB + 4], F16)
            osb = pool.tile([64 + IB, JB], F16)
            nc.sync.dma_start(gsb[64:64 + IB, :], gray_d[:])
            ab = gsb[64:64 + IB, JB:JB + 4].bitcast(F32)
            nc.vector.tensor_scalar(osb[64:64 + IB, :],
                                    gsb[64:64 + IB, 0:JB],
                                    ab[:, 0:1], ab[:, 1:2],
                                    ALU.mult, ALU.add)
            nc.sync.dma_start(out_d[:], osb[64:64 + IB, :])
    nc.finalize()
    return nc


_CACHE = {}


def _get_program(geom):
    key = (geom["P"], geom["NB"])
    if key not in _CACHE:
        _CACHE[key] = _build_nc(geom["P"], geom["NB"])
    return _CACHE[key]


def _in_maps(image3d, geom):
    vol = np.asarray(image3d, np.float64)[0, 0]           # [z, y, x]
    volp = np.concatenate([vol, np.zeros((1, GRID, GRID))], axis=0)
    P, NB = geom["P"], geom["NB"]
    z0 = np.asarray(geom["z0"])
    W = NX + IB
    maps = []
    for c in range(N_CORES):
        r, cb = c // 2, c % 2
        i0, j0 = r * IB, cb * JB
        ylo = geom["row_wins"][r]
        xlo = geom["col_wins"][cb]
        # blended vol slices [P, NY, NX] + interp [P, NY, IB] side by side
        blend = (geom["wz0"][:, None, None]
                 * volp[z0, ylo:ylo + NY, xlo:xlo + NX]
                 + geom["wz1"][:, None, None]
                 * volp[z0 + 1, ylo:ylo + NY, xlo:xlo + NX])
        at1 = (geom["Ay"][:, ylo:ylo + NY, i0:i0 + IB]
               * geom["a_scale"][:, None, i0:i0 + IB])
        va = np.concatenate([blend, at1], axis=2)         # [P, NY, W]
        va_c = np.ascontiguousarray(
            va.transpose(1, 0, 2).reshape(NY, P * W)).astype(np.float16)
        bx = geom["Bx"][:, xlo:xlo + NX, j0:j0 + JB]
        bt = bx * geom["b_scale"][:, None, j0:j0 + JB]
        bt_c = np.ascontiguousarray(
            bt.transpose(1, 0, 2).reshape(NX, P * JB)).astype(np.float16)
        vb = np.concatenate(
            [geom["v"][:, i0:i0 + IB, j0:j0 + JB].transpose(1, 0, 2)
                 .reshape(IB, NB * JB),
             geom["opac4"][i0:i0 + IB, j0:j0 + JB]], axis=1)
        vb_c = np.ascontiguousarray(vb).astype(np.float16)
        maps.append({"va": va_c, "bt": bt_c, "vb": vb_c})
    return maps


def run_kernel(image3d, R, T, trace=False):
    geom = _host_geometry(R, T)
    nc = _get_program(geom)
    maps = _in_maps(image3d, geom)
    res = run_bass_kernel_spmd(nc, maps, list(range(N_CORES)), trace=trace)
    stats = np.stack(
        [np.ascontiguousarray(res.results[c]["out"][:, JB:JB + 4])
             .view(np.float32) for c in range(N_CORES)])
    gmin = float(stats[:, :, 0].min())
    gmax = float(stats[:, :, 1].max())
    a = 1.0 / (gmax - gmin)
    b = -gmin * a
    ab64 = np.tile(np.array([[a, b]], np.float32).view(np.float16), (IB, 1))
    if "affine" not in _CACHE:
        _CACHE["affine"] = _build_affine()
    nc2 = _CACHE["affine"]
    maps2 = []
    for c in range(N_CORES):
        g = np.array(res.results[c]["out"], np.float16)
        g[:, JB:JB + 4] = ab64
        maps2.append({"gray": g})
    res2 = run_bass_kernel_spmd(nc2, maps2, list(range(N_CORES)), trace=trace)
    out = np.zeros((1, 1, IMG_H, IMG_W), np.float32)
    for c in range(N_CORES):
        i0 = (c // 2) * IB
        j0 = (c % 2) * JB
        out[0, 0, i0:i0 + IB, j0:j0 + JB] = res2.results[c]["out"]
    return out, (res, res2)


def kernel(image3d, R, T):
    out, _ = run_kernel(image3d, R, T, trace=False)
    return out


# revision 33
# speedup vs baseline: 1.0065x; 1.0065x over previous
"""Trainium2 Bass kernel for BaseXRayVolumeRenderer.

Full-input contract: kernel(**inputs) takes the unsharded inputs and returns
the full [1,1,256,256] output. Internally shards the 256x256 pixel grid
across 8 NeuronCores (4 row-blocks x 2 col-blocks).

Math: with R = I the trilinear sampling is separable per depth sample p:
    S_p = A_p^T @ (wz0*vol[z0] + wz1*vol[z1]) @ B_p
The z-blend is host-precomputed per depth sample (z0 is strictly increasing,
so each (z0, z0+1) slice pair belongs to exactly one p); the blended slice
and the A_p interp matrix are packed side by side in one "va" tensor, so
stage 1 is a single K=42 matmul per sample and stage 2 a single K=65 matmul
accumulating in PSUM.  Emission-absorption weights factorize into diagonal
scalings folded into A (sy/192) and B (sx), plus a per-block-of-8 rank-1
term G_p ~= u_p * v_b (u folded into B):
    gray = opac/4 + sum_b v_b * pacc_b.

Layout: frustum slicing - each core only loads the vol rows/cols its rays
touch (ny=42 of 128 y-rows, nx=65 of 128 x-cols): ~1.95MB HBM per core in
11 wave DMAs over three queues (vs 8.4MB in 28 DMAs for the two-matmul
z-gather layout).  Depth samples are processed in groups of 4 sharing one
PSUM tile; PSUM->SBUF f16 copies alternate between the vector and scalar
engines.

The global standardize+normalize reduces to out = (gray-gmin)/(gmax-gmin)
(the reference's 1e-8-epsilon terms contribute O(1e-9)).  In-kernel
AllReduce costs ~70us and remote_dma crashes on this platform, so per-core
per-row min/max go to the host, which combines 8x64 values and launches a
tiny second NEFF applying the affine to the f16 gray handoff.
"""

import numpy as np

import concourse.bass as bass
import concourse.bacc as bacc
import concourse.mybir as mybir
import concourse.tile as tile
from concourse.bass_utils import run_bass_kernel_spmd

F32 = mybir.dt.float32
F16 = mybir.dt.float16
F8 = mybir.dt.float8e4
ALU = mybir.AluOpType

IMG_H = 256
IMG_W = 256
N_PTS = 192
MIN_DEPTH, MAX_DEPTH, FOCAL = 3.0, 9.0, 4.0
EPS, EA_EPS = 1e-8, 1e-10
GRID = 128
N_CORES = 8
IB, JB = 64, 128            # per-core pixel block: 64 rows x 128 cols
BS = 8                      # depth-block size for the rank-1 absorption
NY, NX = 42, 65             # per-core vol window (y rows, x cols)
PB = 0                      # base partition for va/bt/py/ysb tiles
WARM_MM = 14                # warmup matmuls (cover first DMA wave, warm HAM)
P_WAVES = (0, 2, 8, 20, 40, 65)  # p-ranges of the DMA waves (tiny first)


def _interp_matrix(f):
    """f: [P, M] voxel coords -> [P, GRID, M] relu(1-|f-k|) interp weights."""
    k = np.arange(GRID, dtype=np.float64)[None, :, None]
    return np.maximum(0.0, 1.0 - np.abs(f[:, None, :] - k))


def _host_geometry(R, T):
    R = np.asarray(R, np.float64)
    T = np.asarray(T, np.float64)[0]
    assert np.allclose(R[0], np.eye(3), atol=1e-5), "kernel assumes R == I"
    ys = np.linspace(1.0, -1.0, IMG_H)
    xs = np.linspace(1.0, -1.0, IMG_W)
    d = np.linspace(MIN_DEPTH, MAX_DEPTH, N_PTS)
    fx = ((xs[None, :] * d[:, None] / FOCAL - T[0]) + 1.0) * 0.5 * (GRID - 1)
    fy = ((ys[None, :] * d[:, None] / FOCAL - T[1]) + 1.0) * 0.5 * (GRID - 1)
    fz = ((d - T[2]) + 1.0) * 0.5 * (GRID - 1)
    zf = np.floor(fz)
    wz = fz - zf
    z0 = np.clip(zf, 0, GRID - 1).astype(np.int64)
    wz0 = (1.0 - wz) * ((zf >= 0) & (zf <= GRID - 1))
    wz1 = wz * ((zf + 1 >= 0) & (zf + 1 <= GRID - 1))
    sz = wz0 + wz1
    active = np.nonzero(sz > 0)[0]
    assert len(active) and active[0] == 0 and np.all(np.diff(active) == 1), \
        "active depth samples must be a prefix for the prefix-cumprod fold"
    P = len(active)
    assert np.all(np.diff(z0[:P]) >= 1), "blend assumes strictly increasing z0"
    Ay = _interp_matrix(fy)[:P]          # [P, 128y, 256i]
    Bx = _interp_matrix(fx)[:P]          # [P, 128x, 256j]
    sy = Ay.sum(axis=1)                  # [P, 256]
    sx = Bx.sum(axis=1)
    dens = (sy[:, :, None] * sx[:, None, :]) * (sz[:P, None, None] / N_PTS)
    t = (1.0 + EA_EPS) - dens
    cp = np.cumprod(t, axis=0)
    absorption = np.concatenate([np.ones_like(cp[:1]), cp[:-1]], axis=0)
    opac4 = 0.25 * (1.0 - np.prod(1.0 - dens, axis=0))  # [H, W]
    # G_p = 0.75*sz_p*absorption_p ~= u_p * v_b  (rank-1 per block of BS)
    G = (0.75 * sz[:P, None, None] * absorption).reshape(P, -1)
    NB = (P + BS - 1) // BS
    u = np.zeros(P)
    v = np.zeros((NB, IMG_H * IMG_W))
    for b in range(NB):
        s, e = b * BS, min((b + 1) * BS, P)
        Ub, Sb, Vb = np.linalg.svd(G[s:e], full_matrices=False)
        sgn = np.sign(Ub[:, 0].mean()) or 1.0
        u[s:e] = Ub[:, 0] * Sb[0] * sgn
        v[b] = Vb[0] * sgn
    a_scale = sy / N_PTS                                  # [P, 256] (i)
    b_scale = sx * u[:, None]                             # [P, 256] (j)
    # per-block vol windows (rows: 4 blocks of 64, cols: 2 blocks of 128)
    row_wins, col_wins = [], []
    for r in range(4):
        nz = np.nonzero(Ay[:, :, r * IB:(r + 1) * IB].sum(axis=(0, 2)) > 0)[0]
        lo = min(int(nz[0]), GRID - NY)
        assert int(nz[-1]) < lo + NY
        row_wins.append(lo)
    for c in range(2):
        nz = np.nonzero(Bx[:, :, c * JB:(c + 1) * JB].sum(axis=(0, 2)) > 0)[0]
        lo = min(int(nz[0]), GRID - NX)
        assert int(nz[-1]) < lo + NX
        col_wins.append(lo)
    return dict(P=P, NB=NB, Ay=Ay, Bx=Bx, z0=[int(z) for z in z0[:P]],
                wz0=wz0[:P], wz1=wz1[:P], a_scale=a_scale, b_scale=b_scale,
                v=v.reshape(NB, IMG_H, IMG_W), opac4=opac4,
                row_wins=row_wins, col_wins=col_wins)


def _build_nc(P, NB):
    """Build the SPMD Bass program (geometry-independent: host pre-blends)."""
    nc = bacc.Bacc(num_devices=N_CORES)
    W = NX + IB                           # 129 cols per p in va
    va_d = nc.declare_dram_parameter("va", [NY, P * W], F16, isOutput=False)
    bt_d = nc.declare_dram_parameter("bt", [NX, P * JB], F16, isOutput=False)
    vb_d = nc.declare_dram_parameter("vb", [IB, (NB + 1) * JB], F16, isOutput=False)
    out_d = nc.declare_dram_parameter("out", [IB, JB + 4], F16, isOutput=True)

    with tile.TileContext(nc) as tc:
        with tc.tile_pool(name="big", bufs=1) as big:
            # partition placement: partitions 0..63 map to the 8 even SDMA
            # engines, 64..127 to the 8 odd ones. bt (stage-2 rhs, K=65,
            # forced to base 0) rides the evens; va and all pixel-row
            # tensors sit at base 64 so their DMAs ride the odds.
            va_sb = big.tile([64 + NY, P * W], F16)
            bt_sb = big.tile([NX, P * JB], F16)
            vb_sb = big.tile([64 + IB, (NB + 1) * JB], F16)
            gray_t = big.tile([64 + IB, JB], F32)
            gray = gray_t[64:64 + IB, :]
            gray16_t = big.tile([64 + IB, JB + 4], F16)
            warm = big.tile([IB, IB], F16)

            nc.vector.memset(warm[:], 0.5)

            # --- streamed loads: 4 waves x (va, bt) alternating HWDGE rings;
            # vb on the SWDGE ring.
            bt_rings = (nc.sync, nc.scalar, nc.sync, nc.scalar, nc.sync)
            va_rings = (nc.scalar, nc.sync, nc.gpsimd, nc.gpsimd, nc.scalar)
            for w in range(len(P_WAVES) - 1):
                p0, p1 = P_WAVES[w], P_WAVES[w + 1]
                va_rings[w].dma_start(va_sb[64:64 + NY, p0 * W:p1 * W],
                                      va_d[:, p0 * W:p1 * W])
                bt_rings[w].dma_start(bt_sb[:, p0 * JB:p1 * JB],
                                      bt_d[:, p0 * JB:p1 * JB])
            # vb is only needed from the first fold on; keep it behind the
            # early va waves in the SWDGE queue's FIFO
            nc.gpsimd.dma_start(vb_sb[64:64 + IB, :], vb_d[:])

            # --- warmup matmuls: PE busy while wave 0 lands (HAM warm-up)
            with tc.tile_pool(name="psW", bufs=1, space="PSUM") as psW:
                wacc = psW.tile([IB, IB], F32)
                for _ in range(WARM_MM):
                    nc.tensor.matmul(wacc[:], warm[:], warm[:],
                                     start=True, stop=True)

            # --- main loop: groups of 4 depth samples share one PSUM tile.
            with tc.tile_pool(name="psY", bufs=3, space="PSUM") as psY, \
                 tc.tile_pool(name="psAcc", bufs=1, space="PSUM") as psAcc, \
                 tc.tile_pool(name="work", bufs=6) as work:
                nacc = (NB + 3) // 4
                pacct = [psAcc.tile([64 + IB, min(4, NB - 4 * a) * JB], F32,
                                    name=f"pacc{a}") for a in range(nacc)]
                paccs = [t[64:64 + IB, :] for t in pacct]

                def fold_block(b):
                    # gray += v_b * pacc_b (gray starts as opac/4); the
                    # final fold writes f16 straight into the output tile
                    pacc = paccs[b // 4]
                    col = (b % 4) * JB
                    tmp = work.tile([64 + IB, JB], F32, tag=f"tmp{b % 2}",
                                    name=f"tmpb{b}")
                    nc.vector.tensor_mul(tmp[64:64 + IB, :],
                                         pacc[:, col:col + JB],
                                         vb_sb[64:64 + IB,
                                               b * JB:(b + 1) * JB])
                    dst = gray16_t[64:64 + IB, 0:JB] if b == NB - 1 else gray
                    nc.vector.tensor_add(dst, gray, tmp[64:64 + IB, :])

                nc.vector.tensor_copy(gray, vb_sb[64:64 + IB, NB * JB:])
                groups = [list(range(s, min(s + 4, P))) for s in range(0, P, 4)]
                for gi, grp in enumerate(groups):
                    py = psY.tile([NX, len(grp) * IB], F32, tag="py",
                                  name=f"py{gi}")
                    for k, kk in enumerate(grp):
                        nc.tensor.matmul(
                            py[:, k * IB:(k + 1) * IB],
                            va_sb[64:64 + NY, kk * W:kk * W + NX],
                            va_sb[64:64 + NY, kk * W + NX:(kk + 1) * W],
                            start=True, stop=True,
                            tile_position=(64, 0))
                    ysb = work.tile([NX, len(grp) * IB], F16, tag="ysb",
                                    name=f"ysb{gi}")
                    if gi < 2 or gi % 2 == 1:
                        nc.vector.tensor_copy(ysb[:], py[:])
                    else:
                        nc.scalar.copy(ysb[:], py[:])
                    for k, kk in enumerate(grp):
                        b = kk // BS
                        pacc = paccs[b // 4]
                        col = (b % 4) * JB
                        first = (kk == b * BS)
                        last = (kk == min((b + 1) * BS, P) - 1)
                        nc.tensor.matmul(pacc[:, col:col + JB],
                                         ysb[:, k * IB:(k + 1) * IB],
                                         bt_sb[:, kk * JB:(kk + 1) * JB],
                                         start=first, stop=last,
                                         tile_position=(0, 64))
                        if last:
                            fold_block(b)

            # --- per-core per-row stats ride in 4 extra f16 columns of the
            # output (bit-cast f32 pairs); host reduces 8x64 values.
            with tc.tile_pool(name="st", bufs=1) as st:
                rowmm = st.tile([64 + IB, 2], F32)
                nc.vector.tensor_reduce(rowmm[64:64 + IB, 0:1],
                                        gray16_t[64:64 + IB, 0:JB],
                                        axis=mybir.AxisListType.X, op=ALU.min)
                nc.vector.tensor_reduce(rowmm[64:64 + IB, 1:2],
                                        gray16_t[64:64 + IB, 0:JB],
                                        axis=mybir.AxisListType.X, op=ALU.max)
                nc.vector.tensor_copy(gray16_t[64:64 + IB, JB:JB + 4],
                                      rowmm[64:64 + IB, :].bitcast(F16))
                nc.sync.dma_start(out_d[:], gray16_t[64:64 + IB, :])
    nc.finalize()
    return nc


def _build_affine():
    """Tiny second NEFF: out = a*gray + b per pixel (a,b host-reduced)."""
    nc = bacc.Bacc(num_devices=N_CORES)
    gray_d = nc.declare_dram_parameter("gray", [IB, JB + 4], F16, isOutput=False)
    out_d = nc.declare_dram_parameter("out", [IB, JB], F16, isOutput=True)
    with tile.TileContext(nc) as tc:
        with tc.tile_pool(name="aff", bufs=1) as pool:
            gsb = pool.tile([64 + IB, JB + 4], F16)
            osb = pool.tile([64 + IB, JB], F16)
            nc.sync.dma_start(gsb[64:64 + IB, :], gray_d[:])
            ab = gsb[64:64 + IB, JB:JB + 4].bitcast(F32)
            nc.vector.tensor_scalar(osb[64:64 + IB, :],
                                    gsb[64:64 + IB, 0:JB],
                                    ab[:, 0:1], ab[:, 1:2],
                                    ALU.mult, ALU.add)
            nc.sync.dma_start(out_d[:], osb[64:64 + IB, :])
    nc.finalize()
    return nc


_CACHE = {}


def _get_program(geom):
    key = (geom["P"], geom["NB"])
    if key not in _CACHE:
        _CACHE[key] = _build_nc(geom["P"], geom["NB"])
    return _CACHE[key]


def _in_maps(image3d, geom):
    vol = np.asarray(image3d, np.float64)[0, 0]           # [z, y, x]
    volp = np.concatenate([vol, np.zeros((1, GRID, GRID))], axis=0)
    P, NB = geom["P"], geom["NB"]
    z0 = np.asarray(geom["z0"])
    W = NX + IB
    maps = []
    for c in range(N_CORES):
        r, cb = c // 2, c % 2
        i0, j0 = r * IB, cb * JB
        ylo = geom["row_wins"][r]
        xlo = geom["col_wins"][cb]
        # blended vol slices [P, NY, NX] + interp [P, NY, IB] side by side
        blend = (geom["wz0"][:, None, None]
                 * volp[z0, ylo:ylo + NY, xlo:xlo + NX]
                 + geom["wz1"][:, None, None]
                 * volp[z0 + 1, ylo:ylo + NY, xlo:xlo + NX])
        at1 = (geom["Ay"][:, ylo:ylo + NY, i0:i0 + IB]
               * geom["a_scale"][:, None, i0:i0 + IB])
        va = np.concatenate([blend, at1], axis=2)         # [P, NY, W]
        va_c = np.ascontiguousarray(
            va.transpose(1, 0, 2).reshape(NY, P * W)).astype(np.float16)
        bx = geom["Bx"][:, xlo:xlo + NX, j0:j0 + JB]
        bt = bx * geom["b_scale"][:, None, j0:j0 + JB]
        bt_c = np.ascontiguousarray(
            bt.transpose(1, 0, 2).reshape(NX, P * JB)).astype(np.float16)
        vb = np.concatenate(
            [geom["v"][:, i0:i0 + IB, j0:j0 + JB].transpose(1, 0, 2)
                 .reshape(IB, NB * JB),
             geom["opac4"][i0:i0 + IB, j0:j0 + JB]], axis=1)
        vb_c = np.ascontiguousarray(vb).astype(np.float16)
        maps.append({"va": va_c, "bt": bt_c, "vb": vb_c})
    return maps


def run_kernel(image3d, R, T, trace=False):
    geom = _host_geometry(R, T)
    nc = _get_program(geom)
    maps = _in_maps(image3d, geom)
    res = run_bass_kernel_spmd(nc, maps, list(range(N_CORES)), trace=trace)
    stats = np.stack(
        [np.ascontiguousarray(res.results[c]["out"][:, JB:JB + 4])
             .view(np.float32) for c in range(N_CORES)])
    gmin = float(stats[:, :, 0].min())
    gmax = float(stats[:, :, 1].max())
    a = 1.0 / (gmax - gmin)
    b = -gmin * a
    ab64 = np.tile(np.array([[a, b]], np.float32).view(np.float16), (IB, 1))
    if "affine" not in _CACHE:
        _CACHE["affine"] = _build_affine()
    nc2 = _CACHE["affine"]
    maps2 = []
    for c in range(N_CORES):
        g = np.array(res.results[c]["out"], np.float16)
        g[:, JB:JB + 4] = ab64
        maps2.append({"gray": g})
    res2 = run_bass_kernel_spmd(nc2, maps2, list(range(N_CORES)), trace=trace)
    out = np.zeros((1, 1, IMG_H, IMG_W), np.float32)
    for c in range(N_CORES):
        i0 = (c // 2) * IB
        j0 = (c % 2) * JB
        out[0, 0, i0:i0 + IB, j0:j0 + JB] = res2.results[c]["out"]
    return out, (res, res2)


def kernel(image3d, R, T):
    out, _ = run_kernel(image3d, R, T, trace=False)
    return out
